# revision 36
# baseline (speedup 1.0000x reference)
"""GroupAttention sparse-attention kernel for 8 trn2 NeuronCores.

Math (derived + numerically verified against the reference):
  - The (a+c) mask keeps only tridiagonal scores -> softmax rows have >=1
    finite entries at j=i+-1, or are fully uniform 1/S ("caseB" rows, where
    eos[i-1]=eos[i+1]=0).
  - neibor = v0 + (vBB-v0)*u u^T  (rank-1 over caseB flags u), overwritten on
    the sub/super diagonals with d_sup (the diagonal needs NO fix: the rank-1
    value there already equals d_main).
  - g[i,j] = exp(cum[j]-cum[i]) for j>i (symmetric), diag d_main, +1e-9
    off-diag (realized as max(g,1e-9): exp underflow land exactly on 1e-9),
    where cum = exclusive prefix-sum of ell=log(d_sup+1e-9).
  - band scores use M = wk^T wq (host-precomputed):
        z = xn @ M^T-ish:  zb[e,i] = sum_f M[f,e] xn[i,f]
        s_next[i] = sum_e xn[i,e] zb[e,i+1],  s_prev[i] = sum_e xn[i,e] zb[e,i-1]
SPMD: one program "compute rows 0..1023". core 2b -> batch b as-is;
core 2b+1 -> batch b with rows reversed (problem is reversal-covariant),
host un-reverses its output half. bq/bk/beta are zeros and gamma ones per the
problem spec, so they are folded away.

Perf notes vs the previous version:
  - no gpsimd compute ops (the DVE<->GpSimd SBUF port lock made every
    concurrent [128,2048] op take ~30us);
  - A~ = wk^T wq computed on host (saves ~1/3 of PE time + weight loads);
  - neibor main tiles depend only on eos_mask+prior -> computed and written
    while the PE crunches the z matmul;
  - band diagonals patched in SBUF with masked adds (no DRAM->DRAM
    per-element DMAs);
  - g generated with fused Exp(scale*x+bias) activations, +1e-9 via ts_max.
"""

import numpy as np
from contextlib import ExitStack

B, S, D = 4, 2048, 1024
NT = 8          # 128-row blocks per core (half of S/128)
HALF = S // 2
WB = 130        # nb band window width

_cache = {}

C_SQ9 = float(np.sqrt(np.float32(1e-9)))                    # sqrt(1e-9)
C_SBB = float(np.sqrt(np.float32((1.0 / S) ** 2 + 1e-9)))   # caseB diag sqrt


def _build():
    import concourse.bass as bass
    import concourse.bacc as bacc
    import concourse.mybir as mybir
    from concourse.tile import TileContext

    f32 = mybir.dt.float32
    bf16 = mybir.dt.bfloat16
    i32 = mybir.dt.int32
    AF = mybir.ActivationFunctionType
    OP = mybir.AluOpType

    f8 = mybir.dt.float8e4
    nc = bacc.Bacc("TRN2", target_bir_lowering=False)

    # ---------------- I/O ----------------
    x_in = nc.dram_tensor("x", [S, D], bf16, kind="ExternalInput")
    at_in = nc.dram_tensor("at", [128, 8, D], bf16, kind="ExternalInput")
    hn_in = nc.dram_tensor("hn", [S], i32, kind="ExternalInput")
    hp_in = nc.dram_tensor("hp", [S], i32, kind="ExternalInput")
    uvec_in = nc.dram_tensor("uvec", [S], f32, kind="ExternalInput")
    cbi_in = nc.dram_tensor("cbi", [S], i32, kind="ExternalInput")
    cbs_in = nc.dram_tensor("cbs", [S], f32, kind="ExternalInput")
    wpv_in = nc.dram_tensor("wpv", [S], f32, kind="ExternalInput")
    wmv_in = nc.dram_tensor("wmv", [S], f32, kind="ExternalInput")
    ucol_in = nc.dram_tensor("ucol", [128, 8], f32, kind="ExternalInput")
    dmcol_in = nc.dram_tensor("dmcol", [128, 8], f32, kind="ExternalInput")
    cvec_in = nc.dram_tensor("cvec", [128, 4], f32, kind="ExternalInput")
    lt_in = nc.dram_tensor("lt128", [128, 128], f32, kind="ExternalInput")
    eye_in = nc.dram_tensor("eye128", [128, 128], f32, kind="ExternalInput")
    bm_in = nc.dram_tensor("bmasks", [4, 128, WB], f32, kind="ExternalInput")
    ones_in = nc.dram_tensor("onesb", [128, 1], bf16, kind="ExternalInput")
    out_nb = nc.dram_tensor("out_nb", [HALF, S], f32, kind="ExternalOutput")
    out_g = nc.dram_tensor("out_g", [HALF, S], f32, kind="ExternalOutput")

    with TileContext(nc) as tc, ExitStack() as ctx:
        # ---------------- pools (whole-kernel lifetime) ----------------
        consts = ctx.enter_context(tc.tile_pool(name="consts", bufs=1))
        big = ctx.enter_context(tc.tile_pool(name="big", bufs=1))
        vec = ctx.enter_context(tc.tile_pool(name="vec", bufs=30))
        xnt_pool = ctx.enter_context(tc.tile_pool(name="xntp", bufs=1))
        zb_pool = ctx.enter_context(tc.tile_pool(name="zbp", bufs=1))
        dram = ctx.enter_context(tc.tile_pool(name="dram", bufs=1, space="DRAM"))
        nbpool = ctx.enter_context(tc.tile_pool(name="nbpool", bufs=2))
        atp = ctx.enter_context(tc.tile_pool(name="atp", bufs=1))
        ptp = ctx.enter_context(tc.tile_pool(name="ptp", bufs=1))
        p2bstack = ExitStack()
        pszp = p2bstack.enter_context(
            tc.tile_pool(name="pszp", bufs=4, space="PSUM")
        )
        p1pools = ExitStack()
        xpool = p1pools.enter_context(tc.tile_pool(name="xpool", bufs=2))
        xbpool = p1pools.enter_context(tc.tile_pool(name="xbpool", bufs=2))
        stpool = p1pools.enter_context(tc.tile_pool(name="stpool", bufs=12))

        # first x chunk + weights first: nothing should queue ahead of them
        xgs = []
        for g2 in range(8):
            xg = xpool.tile([128, 2, D], bf16, tag="xg", name=f"xg{g2}")
            nc.sync.dma_start(
                out=xg,
                in_=x_in[g2 * 256:(g2 + 1) * 256, :].rearrange(
                    "(t p) e -> p t e", p=128
                ),
            )
            xgs.append(xg)
        at_sb = atp.tile([128, 8, D], bf16)  # at[p,ft,e]=(wk^T wq)[f,e]
        nc.sync.dma_start(out=at_sb, in_=at_in[:, :, :])

        # ---------------- consts into SBUF ----------------
        lt128 = consts.tile([128, 128], f32)
        nc.sync.dma_start(out=lt128, in_=lt_in[:, :])
        eye_sb = consts.tile([128, 128], f32)
        nc.sync.dma_start(out=eye_sb, in_=eye_in[:, :])
        bm_sb = consts.tile([128, 4, WB], f32)
        nc.sync.dma_start(out=bm_sb, in_=bm_in[:, :, :].rearrange("v p w -> p v w"))
        ones_b = consts.tile([128, 1], bf16)
        nc.sync.dma_start(out=ones_b, in_=ones_in[:, :])
        cvec = consts.tile([128, 4], f32)
        nc.sync.dma_start(out=cvec, in_=cvec_in[:, :])
        ucol8 = consts.tile([128, 8], f32)
        nc.sync.dma_start(out=ucol8, in_=ucol_in[:, :])
        dmcol8 = consts.tile([128, 8], f32)
        nc.sync.dma_start(out=dmcol8, in_=dmcol_in[:, :])
        v0c = cvec[:, 0:1]
        prc = cvec[:, 1:2]
        ompc = cvec[:, 2:3]
        # register const bias columns used by activation(bias=float)
        for ci, cval in enumerate((0.0, 1e-9, 1e-5)):
            cc = consts.tile([128, 1], f32, name=f"cc{ci}", tag=f"cc{ci}")
            nc.vector.memset(cc, cval)
            nc.const_aps.aps[(f32, cval)] = cc[:, :]

        # u broadcast row (every partition = full u vector)
        urow = big.tile([128, S], f32)
        nc.sync.dma_start(
            out=urow,
            in_=bass.AP(tensor=uvec_in[:].tensor, offset=uvec_in[:].offset,
                        ap=[[0, 128], [1, S]]),
        )

        # ---------------- DRAM scratch ----------------
        xb_d = dram.tile([S, D], bf16)          # normalized x, bf16
        snext_d = dram.tile([S], f32)
        sprev_d = dram.tile([S], f32)
        cum_d = dram.tile([S], f32)
        dsup_d = dram.tile([S + 1], f32)        # [0]=0, [1+i]=d_sup[i]
        colpack_d = dram.tile([HALF, 4], f32)   # per-row cols: cum,-cum,dSub,dSup

        # big SBUF residents
        xnt = xnt_pool.tile([128, 8, S], bf16)  # xnt[p,ft,i] = xn[i, ft*128+p]
        zball = zb_pool.tile([128, 8, S], bf16)  # zball[p,et,i] = zb[et*128+p, i]

        def emit_nb_tile(nbpool, t):
            r0 = t * 128
            w0 = 0 if t == 0 else r0 - 1
            nbt = nbpool.tile([128, S], f32, tag="nbt", name=f"nb{t}")
            nc.vector.tensor_scalar(
                nbt, urow, ucol8[:, t:t + 1], v0c, OP.mult, OP.add
            )
            if w0 > 0:
                nc.sync.dma_start(out=out_nb[r0:r0 + 128, 0:w0], in_=nbt[:, 0:w0])
            nc.sync.dma_start(
                out=out_nb[r0:r0 + 128, w0 + WB:S], in_=nbt[:, w0 + WB:S]
            )

        # ============ phase 1: LN + cast + transpose halves ============
        with nc.named_scope("p1_ln"):
            for g2 in range(8):
                xg = xgs[g2]
                xbg = xbpool.tile([128, 2, D], bf16, tag="xbg", name=f"xb{g2}")
                for j in range(2):
                    xt = xg[:, j, :]
                    stats = stpool.tile([128, 2, 6], f32)
                    nc.vector.bn_stats(out=stats[:, 0, :], in_=xt[:, 0:512])
                    nc.vector.bn_stats(out=stats[:, 1, :], in_=xt[:, 512:D])
                    mv = stpool.tile([128, 2], f32)
                    nc.vector.bn_aggr(out=mv, in_=stats)
                    sq = stpool.tile([128, 1], f32)
                    nc.scalar.activation(sq, mv[:, 1:2], AF.Sqrt, bias=1e-5)
                    rstd = stpool.tile([128, 1], f32)
                    nc.vector.reciprocal(rstd, sq)
                    nc.vector.tensor_scalar(
                        xbg[:, j, :], xt, mv[:, 0:1], rstd,
                        OP.subtract, OP.mult,
                    )
                nc.sync.dma_start(
                    out=xb_d[g2 * 256:(g2 + 1) * 256, :].rearrange(
                        "(t p) e -> p t e", p=128
                    ),
                    in_=xbg,
                )
                # transpose each 512-row quarter as soon as it's in DRAM so
                # the PE can start the z matmuls early
                if g2 % 2 == 1:
                    q = g2 // 2
                    for ft in range(8):
                        nc.sync.dma_start(
                            out=xnt[:, ft, q * 512:(q + 1) * 512],
                            in_=xb_d[q * 512:(q + 1) * 512,
                                     ft * 128:(ft + 1) * 128],
                            transpose=True,
                        )
            p1pools.close()

        # ============ phase 2b: z matmuls (fp8 DoubleRow, per half) ============
        # Band products are accumulated over e-blocks in SBUF (bf16 adds on
        # the otherwise-idle DVE) so only one small ones-matmul pass remains
        # after the z PSUM pool closes.
        ptsum1 = ptp.tile([128, S], bf16, tag="ptsum1")
        ptsum2 = ptp.tile([128, S], bf16, tag="ptsum2")
        with nc.named_scope("p2b_matmul"):
            nbi = 0
            for q in range(4):
                for et in range(8):
                    psz = pszp.tile([128, 512], f32, tag="psz")
                    for ft in range(8):
                        nc.tensor.matmul(
                            psz,
                            at_sb[:, ft, et * 128:(et + 1) * 128],
                            xnt[:, ft, q * 512:(q + 1) * 512],
                            start=(ft == 0),
                            stop=(ft == 7),
                        )
                    nc.scalar.copy(
                        out=zball[:, et, q * 512:(q + 1) * 512], in_=psz
                    )
                    if q < 3:
                        # nb rank-1 tiles ride along on the idle DVE + DMA
                        if et % 3 == 2 and nbi < NT:
                            emit_nb_tile(nbpool, nbi)
                            nbi += 1
                    else:
                        # zb for this et is now complete -> band products
                        if et == 0:
                            nc.vector.tensor_tensor(
                                ptsum1[:, 0:S - 1], xnt[:, 0, 0:S - 1],
                                zball[:, 0, 1:S], OP.mult,
                            )
                            nc.vector.tensor_tensor(
                                ptsum2[:, 1:S], xnt[:, 0, 1:S],
                                zball[:, 0, 0:S - 1], OP.mult,
                            )
                        else:
                            pt1 = ptp.tile([128, S], bf16, tag="pt1")
                            nc.vector.tensor_tensor(
                                pt1[:, 0:S - 1], xnt[:, et, 0:S - 1],
                                zball[:, et, 1:S], OP.mult,
                            )
                            nc.vector.tensor_tensor(
                                ptsum1[:, 0:S - 1], ptsum1[:, 0:S - 1],
                                pt1[:, 0:S - 1], OP.add,
                            )
                            pt2 = ptp.tile([128, S], bf16, tag="pt2")
                            nc.vector.tensor_tensor(
                                pt2[:, 1:S], xnt[:, et, 1:S],
                                zball[:, et, 0:S - 1], OP.mult,
                            )
                            nc.vector.tensor_tensor(
                                ptsum2[:, 1:S], ptsum2[:, 1:S],
                                pt2[:, 1:S], OP.add,
                            )
                        if nbi < NT:
                            emit_nb_tile(nbpool, nbi)
                            nbi += 1
            for t in range(nbi, NT):
                emit_nb_tile(nbpool, t)
            p2bstack.close()

        # ============ phase 2c: partition reduce of the band products ============
        with nc.named_scope("p2c_reduce"), ExitStack() as p2cs:
            psnp = p2cs.enter_context(
                tc.tile_pool(name="psnp", bufs=1, space="PSUM")
            )
            rowp = p2cs.enter_context(tc.tile_pool(name="rowp", bufs=1))
            ps_n = psnp.tile([1, S], f32, tag="psn", name="ps_n")
            ps_p = psnp.tile([1, S], f32, tag="psp", name="ps_p")
            for c in range(4):
                nc.tensor.matmul(
                    ps_n[0:1, c * 512:(c + 1) * 512],
                    ones_b,
                    ptsum1[:, c * 512:(c + 1) * 512],
                    start=True, stop=True,
                )
            for c in range(4):
                nc.tensor.matmul(
                    ps_p[0:1, c * 512:(c + 1) * 512],
                    ones_b,
                    ptsum2[:, c * 512:(c + 1) * 512],
                    start=True, stop=True,
                )
            row_n = rowp.tile([1, S], f32, tag="rn")
            nc.scalar.mul(row_n, ps_n[0:1, :], 1.0 / 512.0)
            nc.sync.dma_start(out=snext_d[:], in_=row_n)
            row_p = rowp.tile([1, S], f32, tag="rp")
            nc.scalar.mul(row_p, ps_p[0:1, :], 1.0 / 512.0)
            nc.sync.dma_start(out=sprev_d[:], in_=row_p)

        # ============ phase 3: band math in [128,16] layout ============
        def v16(name):
            return vec.tile([128, 16], f32, tag="v16", name=name)

        def rd16(dtensor, off):  # dram vec [off:off+2048] -> [128,16] row-major
            return dtensor[off:off + S].rearrange("(p c) -> p c", c=16)

        with nc.named_scope("p3_band"):
            sn = v16("sn")
            nc.sync.dma_start(out=sn, in_=rd16(snext_d, 0))
            sp = v16("sp")
            nc.sync.dma_start(out=sp, in_=rd16(sprev_d, 0))
            hn_i = vec.tile([128, 16], i32, tag="v16i", name="hn_i")
            nc.sync.dma_start(out=hn_i, in_=rd16(hn_in[:], 0))
            hp_i = vec.tile([128, 16], i32, tag="v16i", name="hp_i")
            nc.sync.dma_start(out=hp_i, in_=rd16(hp_in[:], 0))
            cbi = vec.tile([128, 16], i32, tag="v16i", name="cbi")
            nc.sync.dma_start(out=cbi, in_=rd16(cbi_in[:], 0))
            cbS = v16("cbS")
            nc.sync.dma_start(out=cbS, in_=rd16(cbs_in[:], 0))
            wpv = v16("wpv")
            nc.sync.dma_start(out=wpv, in_=rd16(wpv_in[:], 0))
            wmv = v16("wmv")
            nc.sync.dma_start(out=wmv, in_=rd16(wmv_in[:], 0))
            neg9 = v16("neg9")
            nc.vector.memset(neg9, -1.0e9)

            # scores are tiny (|s| <~ 0.3) so exp never overflows; masked
            # entries are exp(-1e9)=0.  caseB rows would give 0/0 -> add the
            # cb flag to the denominator (their sm value is fixed by the
            # blend below anyway).
            sne = v16("sne")
            nc.vector.select(sne, hn_i, sn, neg9)
            spe = v16("spe")
            nc.vector.select(spe, hp_i, sp, neg9)
            en = v16("en")
            nc.scalar.activation(en, sne, AF.Exp)
            ep = v16("ep")
            nc.scalar.activation(ep, spe, AF.Exp)
            zs = v16("zs")
            nc.vector.tensor_tensor(zs, en, ep, OP.add)
            rz = v16("rz")
            nc.vector.reciprocal(rz, zs)
            # caseB rows give 0*inf=NaN here; the select replaces them
            # with the uniform 1/S value.
            nn = v16("nn")
            nc.vector.tensor_tensor(nn, en, rz, OP.mult)
            nc.vector.select(nn, cbi, cbS, nn)
            npv = v16("npv")
            nc.vector.tensor_tensor(npv, ep, rz, OP.mult)
            nc.vector.select(npv, cbi, cbS, npv)
            # np shifted by +1 (value at i+1)
            npsh = v16("npsh")
            nc.vector.memset(npsh, 0.0)
            nc.vector.tensor_copy(out=npsh[:, 0:15], in_=npv[:, 1:16])
            nc.sync.dma_start(out=npsh[0:127, 15:16], in_=npv[1:128, 0:1])
            msup = v16("msup")
            nc.vector.tensor_tensor(msup, nn, npsh, OP.mult)
            # d_sup = prior + (1-prior)*sqrt(msup+1e-9)
            dsup = v16("dsup")
            nc.scalar.activation(dsup, msup, AF.Sqrt, bias=1e-9)
            nc.vector.tensor_scalar(dsup, dsup, ompc, prc, OP.mult, OP.add)
            # ell, prefix sums
            ell = v16("ell")
            nc.scalar.activation(ell, dsup, AF.Ln, bias=1e-9)
            zv16 = v16("zv16")
            nc.vector.memset(zv16, 0.0)
            incl = v16("incl")
            nc.vector.tensor_tensor_scan(incl, ell, zv16, 0.0, OP.add, OP.add)
            excl = v16("excl")
            nc.vector.tensor_tensor(excl, incl, ell, OP.subtract)
            with ExitStack() as p3s:
                ps3p = p3s.enter_context(
                    tc.tile_pool(name="ps3p", bufs=1, space="PSUM")
                )
                ps3 = ps3p.tile([128, 512], f32)
                nc.tensor.matmul(
                    ps3[:, 0:1], lt128, incl[:, 15:16], start=True, stop=True
                )
                cp_col = vec.tile([128, 1], f32, tag="cpc", name="cp_col")
                nc.vector.tensor_copy(out=cp_col, in_=ps3[:, 0:1])
            cum = v16("cum")
            nc.vector.tensor_scalar(cum, excl, cp_col, None, OP.add)
            ncum = v16("ncum")
            nc.vector.tensor_scalar(ncum, cum, -1.0, None, OP.mult)

            def wr16(dtensor, off, src):
                nc.sync.dma_start(
                    out=dtensor[off:off + S].rearrange("(p c) -> p c", c=16),
                    in_=src,
                )

            wr16(cum_d, 0, cum)
            wr16(dsup_d, 1, dsup)
            z1 = vec.tile([1, 1], f32, tag="z1", name="z1")
            nc.vector.memset(z1, 0.0)
            nc.sync.dma_start(out=dsup_d[0:1], in_=z1)
            dsupsh = v16("dsupsh")   # dsup[i-1]
            nc.sync.dma_start(out=dsupsh, in_=rd16(dsup_d, 0))
            dsub = v16("dsub")       # delta for subdiagonal of nb
            nc.vector.tensor_tensor(dsub, dsupsh, wmv, OP.subtract)
            dsp = v16("dsp")         # delta for superdiagonal of nb
            nc.vector.tensor_tensor(dsp, dsup, wpv, OP.subtract)

            # pack per-row columns -> DRAM -> [128, 8, 4] col-major reload
            pack = vec.tile([128, 16, 4], f32, tag="pack", name="pack")
            nc.vector.tensor_copy(out=pack[:, :, 0], in_=cum)
            nc.vector.tensor_copy(out=pack[:, :, 1], in_=ncum)
            nc.vector.tensor_copy(out=pack[:, :, 2], in_=dsub)
            nc.vector.tensor_copy(out=pack[:, :, 3], in_=dsp)
            nc.sync.dma_start(
                out=colpack_d[0:HALF, :].rearrange("(p c) q -> p c q", p=64),
                in_=pack[0:64, :, :],
            )
            cp_all = big.tile([128, 8, 4], f32)
            nc.sync.dma_start(
                out=cp_all,
                in_=colpack_d[0:HALF, :].rearrange("(t p) q -> p t q", p=128),
            )
            cumrow = big.tile([128, S], f32)
            nc.sync.dma_start(
                out=cumrow,
                in_=bass.AP(tensor=cum_d[:].tensor, offset=cum_d[:].offset,
                            ap=[[0, 128], [1, S]]),
            )

        # ============ phase 4: g tiles + nb band windows ============
        with nc.named_scope("p4_out"):
            with ExitStack() as p4:
                gp = p4.enter_context(tc.tile_pool(name="gp", bufs=3))
                winp = p4.enter_context(tc.tile_pool(name="winp", bufs=3))
                bwp = p4.enter_context(tc.tile_pool(name="bwp", bufs=3))

                for t in range(NT):
                    r0 = t * 128
                    w0 = 0 if t == 0 else r0 - 1
                    cum_c = cp_all[:, t, 0:1]
                    ncum_c = cp_all[:, t, 1:2]
                    dsub_c = cp_all[:, t, 2:3]
                    dsp_c = cp_all[:, t, 3:4]

                    # g tile: exp(cum_j - cum_i) off-diag (cum non-increasing
                    # -> both triangles are exp(-|delta|)), diag dmain via
                    # eye*(dmain-1) add; +1e-9 via max floor (exp underflow
                    # lands exactly on 1e-9).  Ship each region as it's done.
                    g = gp.tile([128, S], f32, tag="g", name=f"g{t}")
                    if t > 0:
                        nc.scalar.activation(
                            g[:, 0:r0], cumrow[:, 0:r0], AF.Exp,
                            bias=cum_c, scale=-1.0,
                        )
                        nc.vector.tensor_scalar(
                            g[:, 0:r0], g[:, 0:r0], 1e-9, None, OP.max
                        )
                        nc.sync.dma_start(
                            out=out_g[r0:r0 + 128, 0:r0], in_=g[:, 0:r0]
                        )
                    nc.scalar.activation(
                        g[:, r0 + 128:S], cumrow[:, r0 + 128:S], AF.Exp,
                        bias=ncum_c, scale=1.0,
                    )
                    nc.vector.tensor_scalar(
                        g[:, r0 + 128:S], g[:, r0 + 128:S], 1e-9, None, OP.max
                    )
                    nc.sync.dma_start(
                        out=out_g[r0:r0 + 128, r0 + 128:S],
                        in_=g[:, r0 + 128:S],
                    )
                    # exp(-|w|) = min(exp(w), exp(-w)); inf from overflow is
                    # harmless under min.
                    w = winp.tile([128, 128], f32, tag="w", name=f"w{t}")
                    nc.vector.tensor_scalar(
                        w, cumrow[:, r0:r0 + 128], cum_c, None, OP.subtract
                    )
                    e1 = winp.tile([128, 128], f32, tag="e1", name=f"e1_{t}")
                    nc.scalar.activation(e1, w, AF.Exp)
                    e2 = winp.tile([128, 128], f32, tag="e2", name=f"e2_{t}")
                    nc.scalar.activation(e2, w, AF.Exp, scale=-1.0)
                    nc.vector.tensor_tensor(
                        g[:, r0:r0 + 128], e1, e2, OP.min
                    )
                    eyed = winp.tile([128, 128], f32, tag="eyed", name=f"ey{t}")
                    nc.vector.tensor_scalar(
                        eyed, eye_sb, dmcol8[:, t:t + 1], None, OP.mult
                    )
                    nc.vector.tensor_tensor(
                        g[:, r0:r0 + 128], g[:, r0:r0 + 128], eyed, OP.add
                    )
                    nc.vector.tensor_scalar(
                        g[:, r0:r0 + 128], g[:, r0:r0 + 128], 1e-9, None,
                        OP.max,
                    )
                    nc.sync.dma_start(
                        out=out_g[r0:r0 + 128, r0:r0 + 128],
                        in_=g[:, r0:r0 + 128],
                    )

                    # nb band window [r0, w0:w0+WB]
                    bw = bwp.tile([128, WB], f32, tag="bw", name=f"bw{t}")
                    nc.vector.tensor_scalar(
                        bw, urow[:, w0:w0 + WB], ucol8[:, t:t + 1], v0c,
                        OP.mult, OP.add,
                    )
                    v = 0 if t == 0 else 1
                    tsub = bwp.tile([128, WB], f32, tag="tsub", name=f"ts{t}")
                    nc.vector.tensor_scalar(
                        tsub, bm_sb[:, 2 * v + 0, :], dsub_c, None, OP.mult
                    )
                    nc.vector.tensor_tensor(bw, bw, tsub, OP.add)
                    tsup = bwp.tile([128, WB], f32, tag="tsup", name=f"tp{t}")
                    nc.vector.tensor_scalar(
                        tsup, bm_sb[:, 2 * v + 1, :], dsp_c, None, OP.mult
                    )
                    nc.vector.tensor_tensor(bw, bw, tsup, OP.add)
                    nc.sync.dma_start(
                        out=out_nb[r0:r0 + 128, w0:w0 + WB], in_=bw
                    )

    nc.compile()
    return nc


def _consts():
    k = np.arange(128)
    lt = (k[:, None] < k[None, :]).astype(np.float32)       # lt[k,p]=k<p
    eye = (k[None, :] == k[:, None]).astype(np.float32)
    # band masks [variant, 128, WB]: variant 0 -> t=0 (w0=0), 1 -> t>0 (w0=r0-1)
    w = np.arange(WB)
    bm = np.zeros((4, 128, WB), np.float32)
    bm[0][(w[None, :] == k[:, None] - 1)] = 1.0   # sub,  t=0 (absent for p=0)
    bm[1][(w[None, :] == k[:, None] + 1)] = 1.0   # sup,  t=0
    bm[2][(w[None, :] == k[:, None])] = 1.0       # sub,  t>0
    bm[3][(w[None, :] == k[:, None] + 2)] = 1.0   # sup,  t>0
    import ml_dtypes
    ones = np.ones((128, 1), dtype=ml_dtypes.bfloat16)
    return lt, eye, bm, ones


def kernel(context, eos_mask, prior, wq, bq, wk, bk, gamma, beta):
    import ml_dtypes
    from concourse.bass_utils import run_bass_kernel_spmd

    if "nc" not in _cache:
        _cache["nc"] = _build()
    nc = _cache["nc"]

    context = np.asarray(context, np.float32)
    eos_mask = np.asarray(eos_mask, np.int32)
    prior = float(np.asarray(prior, np.float32).reshape(-1)[0])
    wq = np.asarray(wq, np.float32)
    wk = np.asarray(wk, np.float32)
    lt, eye, bm, ones = _consts()

    # at[p, ft, e] = (wk^T wq)[ft*128+p, e]
    M = (wk.T @ wq).astype(np.float32)
    at = np.ascontiguousarray(
        M.reshape(8, 128, D).transpose(1, 0, 2)
    ).astype(ml_dtypes.bfloat16)

    p32 = np.float32(prior)
    omp = np.float32(1.0) - p32
    v0 = np.float32(p32 + omp * np.float32(C_SQ9))
    vbb = np.float32(p32 + omp * np.float32(C_SBB))
    dv = np.float32(vbb - v0)
    cvec = np.zeros((128, 4), np.float32)
    cvec[:, 0] = v0
    cvec[:, 1] = p32
    cvec[:, 2] = omp

    in_maps = []
    for c in range(8):
        b, h = c // 2, c % 2
        x = context[b] if h == 0 else context[b][::-1]
        eo = eos_mask[b] if h == 0 else eos_mask[b][::-1]
        hn = np.zeros(S, np.int32)
        hn[:S - 1] = eo[1:]
        hp = np.zeros(S, np.int32)
        hp[1:] = eo[:S - 1]
        cb = ((hn == 0) & (hp == 0)).astype(np.float32)
        cbi = ((hn == 0) & (hp == 0)).astype(np.int32)
        cbs = (cb * np.float32(1.0 / S)).astype(np.float32)
        uscl = (dv * cb).astype(np.float32)
        un = np.zeros(S, np.float32)
        un[:S - 1] = cb[1:]
        up = np.zeros(S, np.float32)
        up[1:] = cb[:S - 1]
        wpv = (v0 + uscl * un).astype(np.float32)
        wmv = (v0 + uscl * up).astype(np.float32)
        dmain = (v0 + dv * cb).astype(np.float32)
        ucol8 = np.ascontiguousarray(uscl[:HALF].reshape(8, 128).T)
        # dmcol carries (dmain - 1): g window diag = exp(0) + (dmain-1)
        dmcol8 = np.ascontiguousarray((dmain[:HALF] - 1.0).reshape(8, 128).T
                                      .astype(np.float32))
        in_maps.append({
            "x": np.ascontiguousarray(x).astype(ml_dtypes.bfloat16),
            "at": at,
            "hn": hn, "hp": hp,
            "uvec": cb, "cbi": cbi, "cbs": cbs,
            "wpv": wpv, "wmv": wmv,
            "ucol": ucol8, "dmcol": dmcol8,
            "cvec": cvec,
            "lt128": lt, "eye128": eye,
            "bmasks": bm, "onesb": ones,
        })

    bkr = run_bass_kernel_spmd(nc, in_maps, core_ids=list(range(8)))
    _cache["last_bkr"] = bkr

    g_out = np.empty((B, S, S), np.float32)
    nb_out = np.empty((B, S, S), np.float32)
    for c in range(8):
        b, h = c // 2, c % 2
        rg = bkr.results[c]["out_g"]
        rn = bkr.results[c]["out_nb"]
        if h == 0:
            g_out[b, :HALF] = rg
            nb_out[b, :HALF] = rn
        else:
            g_out[b, HALF:] = rg[::-1, ::-1]
            nb_out[b, HALF:] = rn[::-1, ::-1]
    return g_out, nb_out


# revision 37
# speedup vs baseline: 1.1027x; 1.1027x over previous
"""GroupAttention sparse-attention kernel for 8 trn2 NeuronCores.

Math (derived + numerically verified against the reference):
  - The (a+c) mask keeps only tridiagonal scores -> softmax rows have >=1
    finite entries at j=i+-1, or are fully uniform 1/S ("caseB" rows, where
    eos[i-1]=eos[i+1]=0).
  - neibor = v0 + (vBB-v0)*u u^T  (rank-1 over caseB flags u), overwritten on
    the sub/super diagonals with d_sup (the diagonal needs NO fix: the rank-1
    value there already equals d_main).
  - g[i,j] = exp(cum[j]-cum[i]) for j>i (symmetric), diag d_main, +1e-9
    off-diag (realized as max(g,1e-9): exp underflow land exactly on 1e-9),
    where cum = exclusive prefix-sum of ell=log(d_sup+1e-9).
  - band scores use M = wk^T wq (host-precomputed):
        z = xn @ M^T-ish:  zb[e,i] = sum_f M[f,e] xn[i,f]
        s_next[i] = sum_e xn[i,e] zb[e,i+1],  s_prev[i] = sum_e xn[i,e] zb[e,i-1]
SPMD: one program "compute rows 0..1023". core 2b -> batch b as-is;
core 2b+1 -> batch b with rows reversed (problem is reversal-covariant),
host un-reverses its output half. bq/bk/beta are zeros and gamma ones per the
problem spec, so they are folded away.

Perf notes vs the previous version:
  - no gpsimd compute ops (the DVE<->GpSimd SBUF port lock made every
    concurrent [128,2048] op take ~30us);
  - A~ = wk^T wq computed on host (saves ~1/3 of PE time + weight loads);
  - neibor main tiles depend only on eos_mask+prior -> computed and written
    while the PE crunches the z matmul;
  - band diagonals patched in SBUF with masked adds (no DRAM->DRAM
    per-element DMAs);
  - g generated with fused Exp(scale*x+bias) activations, +1e-9 via ts_max.
"""

import numpy as np
from contextlib import ExitStack

B, S, D = 4, 2048, 1024
NT = 8          # 128-row blocks per core (half of S/128)
HALF = S // 2
WB = 130        # nb band window width

_cache = {}

C_SQ9 = float(np.sqrt(np.float32(1e-9)))                    # sqrt(1e-9)
C_SBB = float(np.sqrt(np.float32((1.0 / S) ** 2 + 1e-9)))   # caseB diag sqrt


def _build():
    import concourse.bass as bass
    import concourse.bacc as bacc
    import concourse.mybir as mybir
    from concourse.tile import TileContext

    f32 = mybir.dt.float32
    bf16 = mybir.dt.bfloat16
    i32 = mybir.dt.int32
    AF = mybir.ActivationFunctionType
    OP = mybir.AluOpType

    f8 = mybir.dt.float8e4
    nc = bacc.Bacc("TRN2", target_bir_lowering=False)

    # ---------------- I/O ----------------
    x_in = nc.dram_tensor("x", [S, D], bf16, kind="ExternalInput")
    at_in = nc.dram_tensor("at", [128, 8, D], bf16, kind="ExternalInput")
    hn_in = nc.dram_tensor("hn", [S], i32, kind="ExternalInput")
    hp_in = nc.dram_tensor("hp", [S], i32, kind="ExternalInput")
    uvec_in = nc.dram_tensor("uvec", [S], f32, kind="ExternalInput")
    cbi_in = nc.dram_tensor("cbi", [S], i32, kind="ExternalInput")
    cbs_in = nc.dram_tensor("cbs", [S], f32, kind="ExternalInput")
    wpv_in = nc.dram_tensor("wpv", [S], f32, kind="ExternalInput")
    wmv_in = nc.dram_tensor("wmv", [S], f32, kind="ExternalInput")
    ucol_in = nc.dram_tensor("ucol", [128, 8], f32, kind="ExternalInput")
    dmcol_in = nc.dram_tensor("dmcol", [128, 8], f32, kind="ExternalInput")
    cvec_in = nc.dram_tensor("cvec", [128, 4], f32, kind="ExternalInput")
    lt_in = nc.dram_tensor("lt128", [128, 128], f32, kind="ExternalInput")
    eye_in = nc.dram_tensor("eye128", [128, 128], f32, kind="ExternalInput")
    bm_in = nc.dram_tensor("bmasks", [4, 128, WB], f32, kind="ExternalInput")
    ones_in = nc.dram_tensor("onesb", [128, 1], bf16, kind="ExternalInput")
    out_nb = nc.dram_tensor("out_nb", [HALF, S], f32, kind="ExternalOutput")
    out_g = nc.dram_tensor("out_g", [HALF, S], f32, kind="ExternalOutput")

    with TileContext(nc) as tc, ExitStack() as ctx:
        # ---------------- pools (whole-kernel lifetime) ----------------
        consts = ctx.enter_context(tc.tile_pool(name="consts", bufs=1))
        big = ctx.enter_context(tc.tile_pool(name="big", bufs=1))
        vec = ctx.enter_context(tc.tile_pool(name="vec", bufs=30))
        xnt_pool = ctx.enter_context(tc.tile_pool(name="xntp", bufs=1))
        zb_pool = ctx.enter_context(tc.tile_pool(name="zbp", bufs=1))
        dram = ctx.enter_context(tc.tile_pool(name="dram", bufs=1, space="DRAM"))
        nbpool = ctx.enter_context(tc.tile_pool(name="nbpool", bufs=2))
        atp = ctx.enter_context(tc.tile_pool(name="atp", bufs=1))
        ptp = ctx.enter_context(tc.tile_pool(name="ptp", bufs=1))
        p2bstack = ExitStack()
        pszp = p2bstack.enter_context(
            tc.tile_pool(name="pszp", bufs=2, space="PSUM")
        )
        p1pools = ExitStack()
        xpool = p1pools.enter_context(tc.tile_pool(name="xpool", bufs=2))
        xbpool = p1pools.enter_context(tc.tile_pool(name="xbpool", bufs=2))
        stpool = p1pools.enter_context(tc.tile_pool(name="stpool", bufs=12))

        # first x chunk + weights first: nothing should queue ahead of them
        xgs = []
        for g2 in range(8):
            xg = xpool.tile([128, 2, D], bf16, tag="xg", name=f"xg{g2}")
            nc.sync.dma_start(
                out=xg,
                in_=x_in[g2 * 256:(g2 + 1) * 256, :].rearrange(
                    "(t p) e -> p t e", p=128
                ),
            )
            xgs.append(xg)
        at_sb = atp.tile([128, 8, D], bf16)  # at[p,ft,e]=(wk^T wq)[f,e]
        nc.sync.dma_start(out=at_sb, in_=at_in[:, :, :])

        # ---------------- consts into SBUF ----------------
        lt128 = consts.tile([128, 128], f32)
        nc.sync.dma_start(out=lt128, in_=lt_in[:, :])
        eye_sb = consts.tile([128, 128], f32)
        nc.sync.dma_start(out=eye_sb, in_=eye_in[:, :])
        bm_sb = consts.tile([128, 4, WB], f32)
        nc.sync.dma_start(out=bm_sb, in_=bm_in[:, :, :].rearrange("v p w -> p v w"))
        ones_b = consts.tile([128, 1], bf16)
        nc.sync.dma_start(out=ones_b, in_=ones_in[:, :])
        cvec = consts.tile([128, 4], f32)
        nc.sync.dma_start(out=cvec, in_=cvec_in[:, :])
        ucol8 = consts.tile([128, 8], f32)
        nc.sync.dma_start(out=ucol8, in_=ucol_in[:, :])
        dmcol8 = consts.tile([128, 8], f32)
        nc.sync.dma_start(out=dmcol8, in_=dmcol_in[:, :])
        v0c = cvec[:, 0:1]
        prc = cvec[:, 1:2]
        ompc = cvec[:, 2:3]
        # register const bias columns used by activation(bias=float)
        for ci, cval in enumerate((0.0, 1e-9, 1e-5)):
            cc = consts.tile([128, 1], f32, name=f"cc{ci}", tag=f"cc{ci}")
            nc.vector.memset(cc, cval)
            nc.const_aps.aps[(f32, cval)] = cc[:, :]

        # u broadcast row (every partition = full u vector)
        urow = big.tile([128, S], f32)
        nc.sync.dma_start(
            out=urow,
            in_=bass.AP(tensor=uvec_in[:].tensor, offset=uvec_in[:].offset,
                        ap=[[0, 128], [1, S]]),
        )

        # ---------------- DRAM scratch ----------------
        xb_d = dram.tile([S, D], bf16)          # normalized x, bf16
        snext_d = dram.tile([S], f32)
        sprev_d = dram.tile([S], f32)
        cum_d = dram.tile([S], f32)
        dsup_d = dram.tile([S + 1], f32)        # [0]=0, [1+i]=d_sup[i]
        colpack_d = dram.tile([HALF, 4], f32)   # per-row cols: cum,-cum,dSub,dSup

        # big SBUF residents
        xnt = xnt_pool.tile([128, 8, S], bf16)  # xnt[p,ft,i] = xn[i, ft*128+p]
        zball = zb_pool.tile([128, 8, S], bf16)  # zball[p,et,i] = zb[et*128+p, i]

        def emit_nb_tile(nbpool, t):
            r0 = t * 128
            w0 = 0 if t == 0 else r0 - 1
            nbt = nbpool.tile([128, S], f32, tag="nbt", name=f"nb{t}")
            nc.vector.tensor_scalar(
                nbt, urow, ucol8[:, t:t + 1], v0c, OP.mult, OP.add
            )
            if w0 > 0:
                nc.sync.dma_start(out=out_nb[r0:r0 + 128, 0:w0], in_=nbt[:, 0:w0])
            nc.sync.dma_start(
                out=out_nb[r0:r0 + 128, w0 + WB:S], in_=nbt[:, w0 + WB:S]
            )

        # ============ phase 1: LN + cast + transpose halves ============
        with nc.named_scope("p1_ln"):
            for g2 in range(8):
                xg = xgs[g2]
                xbg = xbpool.tile([128, 2, D], bf16, tag="xbg", name=f"xb{g2}")
                for j in range(2):
                    xt = xg[:, j, :]
                    stats = stpool.tile([128, 2, 6], f32)
                    nc.vector.bn_stats(out=stats[:, 0, :], in_=xt[:, 0:512])
                    nc.vector.bn_stats(out=stats[:, 1, :], in_=xt[:, 512:D])
                    mv = stpool.tile([128, 2], f32)
                    nc.vector.bn_aggr(out=mv, in_=stats)
                    sq = stpool.tile([128, 1], f32)
                    nc.scalar.activation(sq, mv[:, 1:2], AF.Sqrt, bias=1e-5)
                    rstd = stpool.tile([128, 1], f32)
                    nc.vector.reciprocal(rstd, sq)
                    nc.vector.tensor_scalar(
                        xbg[:, j, :], xt, mv[:, 0:1], rstd,
                        OP.subtract, OP.mult,
                    )
                nc.sync.dma_start(
                    out=xb_d[g2 * 256:(g2 + 1) * 256, :].rearrange(
                        "(t p) e -> p t e", p=128
                    ),
                    in_=xbg,
                )
                # transpose each half as soon as its xb rows are in DRAM
                if g2 in (3, 7):
                    h = g2 // 4
                    for ft in range(8):
                        nc.sync.dma_start(
                            out=xnt[:, ft, h * 1024:(h + 1) * 1024],
                            in_=xb_d[h * 1024:(h + 1) * 1024,
                                     ft * 128:(ft + 1) * 128],
                            transpose=True,
                        )
            p1pools.close()

        # ============ phase 2b: z matmuls (fp8 DoubleRow, per half) ============
        # Band products are accumulated over e-blocks in SBUF (bf16 adds on
        # the otherwise-idle DVE) so only one small ones-matmul pass remains
        # after the z PSUM pool closes.
        ptsum1 = ptp.tile([128, S], bf16, tag="ptsum1")
        ptsum2 = ptp.tile([128, S], bf16, tag="ptsum2")
        with nc.named_scope("p2b_matmul"):
            nbi = 0
            for h in range(2):
                for et in range(8):
                    psz = pszp.tile([128, 1024], f32, tag="psz")
                    for ft in range(8):
                        lhs = at_sb[:, ft, et * 128:(et + 1) * 128]
                        for c in range(2):
                            off = h * 1024 + c * 512
                            nc.tensor.matmul(
                                psz[:, c * 512:(c + 1) * 512],
                                lhs,
                                xnt[:, ft, off:off + 512],
                                start=(ft == 0),
                                stop=(ft == 7),
                            )
                    nc.scalar.copy(
                        out=zball[:, et, h * 1024:(h + 1) * 1024], in_=psz
                    )
                    if h == 0:
                        # nb rank-1 tiles ride along on the idle DVE + DMA
                        if et % 2 == 1 and nbi < NT:
                            emit_nb_tile(nbpool, nbi)
                            nbi += 1
                    else:
                        # zb for this et is now complete -> band products
                        if et == 0:
                            nc.vector.tensor_tensor(
                                ptsum1[:, 0:S - 1], xnt[:, 0, 0:S - 1],
                                zball[:, 0, 1:S], OP.mult,
                            )
                            nc.vector.tensor_tensor(
                                ptsum2[:, 1:S], xnt[:, 0, 1:S],
                                zball[:, 0, 0:S - 1], OP.mult,
                            )
                        else:
                            pt1 = ptp.tile([128, S], bf16, tag="pt1")
                            nc.vector.tensor_tensor(
                                pt1[:, 0:S - 1], xnt[:, et, 0:S - 1],
                                zball[:, et, 1:S], OP.mult,
                            )
                            nc.vector.tensor_tensor(
                                ptsum1[:, 0:S - 1], ptsum1[:, 0:S - 1],
                                pt1[:, 0:S - 1], OP.add,
                            )
                            pt2 = ptp.tile([128, S], bf16, tag="pt2")
                            nc.vector.tensor_tensor(
                                pt2[:, 1:S], xnt[:, et, 1:S],
                                zball[:, et, 0:S - 1], OP.mult,
                            )
                            nc.vector.tensor_tensor(
                                ptsum2[:, 1:S], ptsum2[:, 1:S],
                                pt2[:, 1:S], OP.add,
                            )
                        if nbi < NT:
                            emit_nb_tile(nbpool, nbi)
                            nbi += 1
            for t in range(nbi, NT):
                emit_nb_tile(nbpool, t)
            p2bstack.close()

        # ============ phase 2c: partition reduce of the band products ============
        with nc.named_scope("p2c_reduce"), ExitStack() as p2cs:
            psnp = p2cs.enter_context(
                tc.tile_pool(name="psnp", bufs=1, space="PSUM")
            )
            rowp = p2cs.enter_context(tc.tile_pool(name="rowp", bufs=1))
            ps_n = psnp.tile([1, S], f32, tag="psn", name="ps_n")
            ps_p = psnp.tile([1, S], f32, tag="psp", name="ps_p")
            for c in range(4):
                nc.tensor.matmul(
                    ps_n[0:1, c * 512:(c + 1) * 512],
                    ones_b,
                    ptsum1[:, c * 512:(c + 1) * 512],
                    start=True, stop=True,
                )
            for c in range(4):
                nc.tensor.matmul(
                    ps_p[0:1, c * 512:(c + 1) * 512],
                    ones_b,
                    ptsum2[:, c * 512:(c + 1) * 512],
                    start=True, stop=True,
                )
            row_n = rowp.tile([1, S], f32, tag="rn")
            nc.scalar.mul(row_n, ps_n[0:1, :], 1.0 / 512.0)
            nc.sync.dma_start(out=snext_d[:], in_=row_n)
            row_p = rowp.tile([1, S], f32, tag="rp")
            nc.scalar.mul(row_p, ps_p[0:1, :], 1.0 / 512.0)
            nc.sync.dma_start(out=sprev_d[:], in_=row_p)

        # ============ phase 3: band math in [128,16] layout ============
        def v16(name):
            return vec.tile([128, 16], f32, tag="v16", name=name)

        def rd16(dtensor, off):  # dram vec [off:off+2048] -> [128,16] row-major
            return dtensor[off:off + S].rearrange("(p c) -> p c", c=16)

        with nc.named_scope("p3_band"):
            sn = v16("sn")
            nc.sync.dma_start(out=sn, in_=rd16(snext_d, 0))
            sp = v16("sp")
            nc.sync.dma_start(out=sp, in_=rd16(sprev_d, 0))
            hn_i = vec.tile([128, 16], i32, tag="v16i", name="hn_i")
            nc.sync.dma_start(out=hn_i, in_=rd16(hn_in[:], 0))
            hp_i = vec.tile([128, 16], i32, tag="v16i", name="hp_i")
            nc.sync.dma_start(out=hp_i, in_=rd16(hp_in[:], 0))
            cbi = vec.tile([128, 16], i32, tag="v16i", name="cbi")
            nc.sync.dma_start(out=cbi, in_=rd16(cbi_in[:], 0))
            cbS = v16("cbS")
            nc.sync.dma_start(out=cbS, in_=rd16(cbs_in[:], 0))
            wpv = v16("wpv")
            nc.sync.dma_start(out=wpv, in_=rd16(wpv_in[:], 0))
            wmv = v16("wmv")
            nc.sync.dma_start(out=wmv, in_=rd16(wmv_in[:], 0))
            neg9 = v16("neg9")
            nc.vector.memset(neg9, -1.0e9)

            # scores are tiny (|s| <~ 0.3) so exp never overflows; masked
            # entries are exp(-1e9)=0.  caseB rows would give 0/0 -> add the
            # cb flag to the denominator (their sm value is fixed by the
            # blend below anyway).
            sne = v16("sne")
            nc.vector.select(sne, hn_i, sn, neg9)
            spe = v16("spe")
            nc.vector.select(spe, hp_i, sp, neg9)
            en = v16("en")
            nc.scalar.activation(en, sne, AF.Exp)
            ep = v16("ep")
            nc.scalar.activation(ep, spe, AF.Exp)
            zs = v16("zs")
            nc.vector.tensor_tensor(zs, en, ep, OP.add)
            rz = v16("rz")
            nc.vector.reciprocal(rz, zs)
            # caseB rows give 0*inf=NaN here; the select replaces them
            # with the uniform 1/S value.
            nn = v16("nn")
            nc.vector.tensor_tensor(nn, en, rz, OP.mult)
            nc.vector.select(nn, cbi, cbS, nn)
            npv = v16("npv")
            nc.vector.tensor_tensor(npv, ep, rz, OP.mult)
            nc.vector.select(npv, cbi, cbS, npv)
            # np shifted by +1 (value at i+1)
            npsh = v16("npsh")
            nc.vector.memset(npsh, 0.0)
            nc.vector.tensor_copy(out=npsh[:, 0:15], in_=npv[:, 1:16])
            nc.sync.dma_start(out=npsh[0:127, 15:16], in_=npv[1:128, 0:1])
            msup = v16("msup")
            nc.vector.tensor_tensor(msup, nn, npsh, OP.mult)
            # d_sup = prior + (1-prior)*sqrt(msup+1e-9)
            dsup = v16("dsup")
            nc.scalar.activation(dsup, msup, AF.Sqrt, bias=1e-9)
            nc.vector.tensor_scalar(dsup, dsup, ompc, prc, OP.mult, OP.add)
            # ell, prefix sums
            ell = v16("ell")
            nc.scalar.activation(ell, dsup, AF.Ln, bias=1e-9)
            zv16 = v16("zv16")
            nc.vector.memset(zv16, 0.0)
            incl = v16("incl")
            nc.vector.tensor_tensor_scan(incl, ell, zv16, 0.0, OP.add, OP.add)
            excl = v16("excl")
            nc.vector.tensor_tensor(excl, incl, ell, OP.subtract)
            with ExitStack() as p3s:
                ps3p = p3s.enter_context(
                    tc.tile_pool(name="ps3p", bufs=1, space="PSUM")
                )
                ps3 = ps3p.tile([128, 512], f32)
                nc.tensor.matmul(
                    ps3[:, 0:1], lt128, incl[:, 15:16], start=True, stop=True
                )
                cp_col = vec.tile([128, 1], f32, tag="cpc", name="cp_col")
                nc.vector.tensor_copy(out=cp_col, in_=ps3[:, 0:1])
            cum = v16("cum")
            nc.vector.tensor_scalar(cum, excl, cp_col, None, OP.add)
            ncum = v16("ncum")
            nc.vector.tensor_scalar(ncum, cum, -1.0, None, OP.mult)

            def wr16(dtensor, off, src):
                nc.sync.dma_start(
                    out=dtensor[off:off + S].rearrange("(p c) -> p c", c=16),
                    in_=src,
                )

            wr16(cum_d, 0, cum)
            wr16(dsup_d, 1, dsup)
            z1 = vec.tile([1, 1], f32, tag="z1", name="z1")
            nc.vector.memset(z1, 0.0)
            nc.sync.dma_start(out=dsup_d[0:1], in_=z1)
            dsupsh = v16("dsupsh")   # dsup[i-1]
            nc.sync.dma_start(out=dsupsh, in_=rd16(dsup_d, 0))
            dsub = v16("dsub")       # delta for subdiagonal of nb
            nc.vector.tensor_tensor(dsub, dsupsh, wmv, OP.subtract)
            dsp = v16("dsp")         # delta for superdiagonal of nb
            nc.vector.tensor_tensor(dsp, dsup, wpv, OP.subtract)

            # pack per-row columns -> DRAM -> [128, 8, 4] col-major reload
            pack = vec.tile([128, 16, 4], f32, tag="pack", name="pack")
            nc.vector.tensor_copy(out=pack[:, :, 0], in_=cum)
            nc.vector.tensor_copy(out=pack[:, :, 1], in_=ncum)
            nc.vector.tensor_copy(out=pack[:, :, 2], in_=dsub)
            nc.vector.tensor_copy(out=pack[:, :, 3], in_=dsp)
            nc.sync.dma_start(
                out=colpack_d[0:HALF, :].rearrange("(p c) q -> p c q", p=64),
                in_=pack[0:64, :, :],
            )
            cp_all = big.tile([128, 8, 4], f32)
            nc.sync.dma_start(
                out=cp_all,
                in_=colpack_d[0:HALF, :].rearrange("(t p) q -> p t q", p=128),
            )
            cumrow = big.tile([128, S], f32)
            nc.sync.dma_start(
                out=cumrow,
                in_=bass.AP(tensor=cum_d[:].tensor, offset=cum_d[:].offset,
                            ap=[[0, 128], [1, S]]),
            )

        # ============ phase 4: g tiles + nb band windows ============
        with nc.named_scope("p4_out"):
            with ExitStack() as p4:
                gp = p4.enter_context(tc.tile_pool(name="gp", bufs=3))
                winp = p4.enter_context(tc.tile_pool(name="winp", bufs=3))
                bwp = p4.enter_context(tc.tile_pool(name="bwp", bufs=3))

                for t in range(NT):
                    r0 = t * 128
                    w0 = 0 if t == 0 else r0 - 1
                    cum_c = cp_all[:, t, 0:1]
                    ncum_c = cp_all[:, t, 1:2]
                    dsub_c = cp_all[:, t, 2:3]
                    dsp_c = cp_all[:, t, 3:4]

                    # g tile: exp(cum_j - cum_i) off-diag (cum non-increasing
                    # -> both triangles are exp(-|delta|)), diag dmain via
                    # eye*(dmain-1) add; +1e-9 via max floor (exp underflow
                    # lands exactly on 1e-9).  Ship each region as it's done.
                    g = gp.tile([128, S], f32, tag="g", name=f"g{t}")
                    if t > 0:
                        nc.scalar.activation(
                            g[:, 0:r0], cumrow[:, 0:r0], AF.Exp,
                            bias=cum_c, scale=-1.0,
                        )
                        nc.vector.tensor_scalar(
                            g[:, 0:r0], g[:, 0:r0], 1e-9, None, OP.max
                        )
                        nc.sync.dma_start(
                            out=out_g[r0:r0 + 128, 0:r0], in_=g[:, 0:r0]
                        )
                    nc.scalar.activation(
                        g[:, r0 + 128:S], cumrow[:, r0 + 128:S], AF.Exp,
                        bias=ncum_c, scale=1.0,
                    )
                    nc.vector.tensor_scalar(
                        g[:, r0 + 128:S], g[:, r0 + 128:S], 1e-9, None, OP.max
                    )
                    nc.sync.dma_start(
                        out=out_g[r0:r0 + 128, r0 + 128:S],
                        in_=g[:, r0 + 128:S],
                    )
                    # exp(-|w|) = min(exp(w), exp(-w)); inf from overflow is
                    # harmless under min.
                    w = winp.tile([128, 128], f32, tag="w", name=f"w{t}")
                    nc.vector.tensor_scalar(
                        w, cumrow[:, r0:r0 + 128], cum_c, None, OP.subtract
                    )
                    e1 = winp.tile([128, 128], f32, tag="e1", name=f"e1_{t}")
                    nc.scalar.activation(e1, w, AF.Exp)
                    e2 = winp.tile([128, 128], f32, tag="e2", name=f"e2_{t}")
                    nc.scalar.activation(e2, w, AF.Exp, scale=-1.0)
                    nc.vector.tensor_tensor(
                        g[:, r0:r0 + 128], e1, e2, OP.min
                    )
                    eyed = winp.tile([128, 128], f32, tag="eyed", name=f"ey{t}")
                    nc.vector.tensor_scalar(
                        eyed, eye_sb, dmcol8[:, t:t + 1], None, OP.mult
                    )
                    nc.vector.tensor_tensor(
                        g[:, r0:r0 + 128], g[:, r0:r0 + 128], eyed, OP.add
                    )
                    nc.vector.tensor_scalar(
                        g[:, r0:r0 + 128], g[:, r0:r0 + 128], 1e-9, None,
                        OP.max,
                    )
                    nc.sync.dma_start(
                        out=out_g[r0:r0 + 128, r0:r0 + 128],
                        in_=g[:, r0:r0 + 128],
                    )

                    # nb band window [r0, w0:w0+WB]
                    bw = bwp.tile([128, WB], f32, tag="bw", name=f"bw{t}")
                    nc.vector.tensor_scalar(
                        bw, urow[:, w0:w0 + WB], ucol8[:, t:t + 1], v0c,
                        OP.mult, OP.add,
                    )
                    v = 0 if t == 0 else 1
                    tsub = bwp.tile([128, WB], f32, tag="tsub", name=f"ts{t}")
                    nc.vector.tensor_scalar(
                        tsub, bm_sb[:, 2 * v + 0, :], dsub_c, None, OP.mult
                    )
                    nc.vector.tensor_tensor(bw, bw, tsub, OP.add)
                    tsup = bwp.tile([128, WB], f32, tag="tsup", name=f"tp{t}")
                    nc.vector.tensor_scalar(
                        tsup, bm_sb[:, 2 * v + 1, :], dsp_c, None, OP.mult
                    )
                    nc.vector.tensor_tensor(bw, bw, tsup, OP.add)
                    nc.sync.dma_start(
                        out=out_nb[r0:r0 + 128, w0:w0 + WB], in_=bw
                    )

    nc.compile()
    return nc


def _consts():
    k = np.arange(128)
    lt = (k[:, None] < k[None, :]).astype(np.float32)       # lt[k,p]=k<p
    eye = (k[None, :] == k[:, None]).astype(np.float32)
    # band masks [variant, 128, WB]: variant 0 -> t=0 (w0=0), 1 -> t>0 (w0=r0-1)
    w = np.arange(WB)
    bm = np.zeros((4, 128, WB), np.float32)
    bm[0][(w[None, :] == k[:, None] - 1)] = 1.0   # sub,  t=0 (absent for p=0)
    bm[1][(w[None, :] == k[:, None] + 1)] = 1.0   # sup,  t=0
    bm[2][(w[None, :] == k[:, None])] = 1.0       # sub,  t>0
    bm[3][(w[None, :] == k[:, None] + 2)] = 1.0   # sup,  t>0
    import ml_dtypes
    ones = np.ones((128, 1), dtype=ml_dtypes.bfloat16)
    return lt, eye, bm, ones


def kernel(context, eos_mask, prior, wq, bq, wk, bk, gamma, beta):
    import ml_dtypes
    from concourse.bass_utils import run_bass_kernel_spmd

    if "nc" not in _cache:
        _cache["nc"] = _build()
    nc = _cache["nc"]

    context = np.asarray(context, np.float32)
    eos_mask = np.asarray(eos_mask, np.int32)
    prior = float(np.asarray(prior, np.float32).reshape(-1)[0])
    wq = np.asarray(wq, np.float32)
    wk = np.asarray(wk, np.float32)
    lt, eye, bm, ones = _consts()

    # at[p, ft, e] = (wk^T wq)[ft*128+p, e]
    M = (wk.T @ wq).astype(np.float32)
    at = np.ascontiguousarray(
        M.reshape(8, 128, D).transpose(1, 0, 2)
    ).astype(ml_dtypes.bfloat16)

    p32 = np.float32(prior)
    omp = np.float32(1.0) - p32
    v0 = np.float32(p32 + omp * np.float32(C_SQ9))
    vbb = np.float32(p32 + omp * np.float32(C_SBB))
    dv = np.float32(vbb - v0)
    cvec = np.zeros((128, 4), np.float32)
    cvec[:, 0] = v0
    cvec[:, 1] = p32
    cvec[:, 2] = omp

    in_maps = []
    for c in range(8):
        b, h = c // 2, c % 2
        x = context[b] if h == 0 else context[b][::-1]
        eo = eos_mask[b] if h == 0 else eos_mask[b][::-1]
        hn = np.zeros(S, np.int32)
        hn[:S - 1] = eo[1:]
        hp = np.zeros(S, np.int32)
        hp[1:] = eo[:S - 1]
        cb = ((hn == 0) & (hp == 0)).astype(np.float32)
        cbi = ((hn == 0) & (hp == 0)).astype(np.int32)
        cbs = (cb * np.float32(1.0 / S)).astype(np.float32)
        uscl = (dv * cb).astype(np.float32)
        un = np.zeros(S, np.float32)
        un[:S - 1] = cb[1:]
        up = np.zeros(S, np.float32)
        up[1:] = cb[:S - 1]
        wpv = (v0 + uscl * un).astype(np.float32)
        wmv = (v0 + uscl * up).astype(np.float32)
        dmain = (v0 + dv * cb).astype(np.float32)
        ucol8 = np.ascontiguousarray(uscl[:HALF].reshape(8, 128).T)
        # dmcol carries (dmain - 1): g window diag = exp(0) + (dmain-1)
        dmcol8 = np.ascontiguousarray((dmain[:HALF] - 1.0).reshape(8, 128).T
                                      .astype(np.float32))
        in_maps.append({
            "x": np.ascontiguousarray(x).astype(ml_dtypes.bfloat16),
            "at": at,
            "hn": hn, "hp": hp,
            "uvec": cb, "cbi": cbi, "cbs": cbs,
            "wpv": wpv, "wmv": wmv,
            "ucol": ucol8, "dmcol": dmcol8,
            "cvec": cvec,
            "lt128": lt, "eye128": eye,
            "bmasks": bm, "onesb": ones,
        })

    bkr = run_bass_kernel_spmd(nc, in_maps, core_ids=list(range(8)))
    _cache["last_bkr"] = bkr

    g_out = np.empty((B, S, S), np.float32)
    nb_out = np.empty((B, S, S), np.float32)
    for c in range(8):
        b, h = c // 2, c % 2
        rg = bkr.results[c]["out_g"]
        rn = bkr.results[c]["out_nb"]
        if h == 0:
            g_out[b, :HALF] = rg
            nb_out[b, :HALF] = rn
        else:
            g_out[b, HALF:] = rg[::-1, ::-1]
            nb_out[b, HALF:] = rn[::-1, ::-1]
    return g_out, nb_out


# revision 40
# speedup vs baseline: 1.1814x; 1.0714x over previous
"""GroupAttention sparse-attention kernel for 8 trn2 NeuronCores.

Math (derived + numerically verified against the reference):
  - The (a+c) mask keeps only tridiagonal scores -> softmax rows have >=1
    finite entries at j=i+-1, or are fully uniform 1/S ("caseB" rows, where
    eos[i-1]=eos[i+1]=0).
  - neibor = v0 + (vBB-v0)*u u^T  (rank-1 over caseB flags u), overwritten on
    the sub/super diagonals with d_sup (the diagonal needs NO fix: the rank-1
    value there already equals d_main).
  - g[i,j] = exp(cum[j]-cum[i]) for j>i (symmetric), diag d_main, +1e-9
    off-diag (realized as max(g,1e-9): exp underflow land exactly on 1e-9),
    where cum = exclusive prefix-sum of ell=log(d_sup+1e-9).
  - band scores use M = wk^T wq (host-precomputed):
        z = xn @ M^T-ish:  zb[e,i] = sum_f M[f,e] xn[i,f]
        s_next[i] = sum_e xn[i,e] zb[e,i+1],  s_prev[i] = sum_e xn[i,e] zb[e,i-1]
SPMD: one program "compute rows 0..1023". core 2b -> batch b as-is;
core 2b+1 -> batch b with rows reversed (problem is reversal-covariant),
host un-reverses its output half. bq/bk/beta are zeros and gamma ones per the
problem spec, so they are folded away.

Perf notes vs the previous version:
  - no gpsimd compute ops (the DVE<->GpSimd SBUF port lock made every
    concurrent [128,2048] op take ~30us);
  - A~ = wk^T wq computed on host (saves ~1/3 of PE time + weight loads);
  - neibor main tiles depend only on eos_mask+prior -> computed and written
    while the PE crunches the z matmul;
  - band diagonals patched in SBUF with masked adds (no DRAM->DRAM
    per-element DMAs);
  - g generated with fused Exp(scale*x+bias) activations, +1e-9 via ts_max.
"""

import numpy as np
from contextlib import ExitStack

B, S, D = 4, 2048, 1024
NT = 8          # 128-row blocks per core (half of S/128)
HALF = S // 2
WB = 130        # nb band window width

_cache = {}

C_SQ9 = float(np.sqrt(np.float32(1e-9)))                    # sqrt(1e-9)
C_SBB = float(np.sqrt(np.float32((1.0 / S) ** 2 + 1e-9)))   # caseB diag sqrt


def _build():
    import concourse.bass as bass
    import concourse.bacc as bacc
    import concourse.mybir as mybir
    from concourse.tile import TileContext

    f32 = mybir.dt.float32
    bf16 = mybir.dt.bfloat16
    i32 = mybir.dt.int32
    AF = mybir.ActivationFunctionType
    OP = mybir.AluOpType

    f8 = mybir.dt.float8e4
    nc = bacc.Bacc("TRN2", target_bir_lowering=False)

    # ---------------- I/O ----------------
    x_in = nc.dram_tensor("x", [S, D], bf16, kind="ExternalInput")
    at_in = nc.dram_tensor("at", [128, 8, D], bf16, kind="ExternalInput")
    hn_in = nc.dram_tensor("hn", [S], i32, kind="ExternalInput")
    hp_in = nc.dram_tensor("hp", [S], i32, kind="ExternalInput")
    uvec_in = nc.dram_tensor("uvec", [S], f32, kind="ExternalInput")
    cbi_in = nc.dram_tensor("cbi", [S], i32, kind="ExternalInput")
    cbs_in = nc.dram_tensor("cbs", [S], f32, kind="ExternalInput")
    wpv_in = nc.dram_tensor("wpv", [S], f32, kind="ExternalInput")
    wmv_in = nc.dram_tensor("wmv", [S], f32, kind="ExternalInput")
    ucol_in = nc.dram_tensor("ucol", [128, 8], f32, kind="ExternalInput")
    dmcol_in = nc.dram_tensor("dmcol", [128, 8], f32, kind="ExternalInput")
    cvec_in = nc.dram_tensor("cvec", [128, 4], f32, kind="ExternalInput")
    lt_in = nc.dram_tensor("lt128", [128, 128], f32, kind="ExternalInput")
    eye_in = nc.dram_tensor("eye128", [128, 128], f32, kind="ExternalInput")
    bm_in = nc.dram_tensor("bmasks", [4, 128, WB], f32, kind="ExternalInput")
    ones_in = nc.dram_tensor("onesb", [128, 1], bf16, kind="ExternalInput")
    out_nb = nc.dram_tensor("out_nb", [HALF, S], f32, kind="ExternalOutput")
    out_g = nc.dram_tensor("out_g", [HALF, S], f32, kind="ExternalOutput")

    with TileContext(nc) as tc, ExitStack() as ctx:
        # ---------------- pools (whole-kernel lifetime) ----------------
        consts = ctx.enter_context(tc.tile_pool(name="consts", bufs=1))
        big = ctx.enter_context(tc.tile_pool(name="big", bufs=1))
        vec = ctx.enter_context(tc.tile_pool(name="vec", bufs=30))
        xnt_pool = ctx.enter_context(tc.tile_pool(name="xntp", bufs=1))
        zb_pool = ctx.enter_context(tc.tile_pool(name="zbp", bufs=1))
        dram = ctx.enter_context(tc.tile_pool(name="dram", bufs=1, space="DRAM"))
        nbpool = ctx.enter_context(tc.tile_pool(name="nbpool", bufs=2))
        atp = ctx.enter_context(tc.tile_pool(name="atp", bufs=1))
        ptp = ctx.enter_context(tc.tile_pool(name="ptp", bufs=1))
        p2bstack = ExitStack()
        pszp = p2bstack.enter_context(
            tc.tile_pool(name="pszp", bufs=2, space="PSUM")
        )
        p1pools = ExitStack()
        xpool = p1pools.enter_context(tc.tile_pool(name="xpool", bufs=3))
        xbpool = p1pools.enter_context(tc.tile_pool(name="xbpool", bufs=3))
        stpool = p1pools.enter_context(tc.tile_pool(name="stpool", bufs=12))
        dpool = p1pools.enter_context(tc.tile_pool(name="dpool", bufs=2))

        # first x chunk + weights first: nothing should queue ahead of them
        xgs = []
        for g2 in range(8):
            xg = xpool.tile([128, 2, D], bf16, tag="xg", name=f"xg{g2}")
            nc.sync.dma_start(
                out=xg,
                in_=x_in[g2 * 256:(g2 + 1) * 256, :].rearrange(
                    "(t p) e -> p t e", p=128
                ),
            )
            xgs.append(xg)
        at_sb = atp.tile([128, 8, D], bf16)  # at[p,ft,e]=(wk^T wq)[f,e]
        nc.sync.dma_start(out=at_sb, in_=at_in[:, :, :])

        # ---------------- consts into SBUF ----------------
        lt128 = consts.tile([128, 128], f32)
        nc.sync.dma_start(out=lt128, in_=lt_in[:, :])
        eye_sb = consts.tile([128, 128], f32)
        nc.sync.dma_start(out=eye_sb, in_=eye_in[:, :])
        bm_sb = consts.tile([128, 4, WB], f32)
        nc.sync.dma_start(out=bm_sb, in_=bm_in[:, :, :].rearrange("v p w -> p v w"))
        ones_b = consts.tile([128, 1], bf16)
        nc.sync.dma_start(out=ones_b, in_=ones_in[:, :])
        cvec = consts.tile([128, 4], f32)
        nc.sync.dma_start(out=cvec, in_=cvec_in[:, :])
        ucol8 = consts.tile([128, 8], f32)
        nc.sync.dma_start(out=ucol8, in_=ucol_in[:, :])
        dmcol8 = consts.tile([128, 8], f32)
        nc.sync.dma_start(out=dmcol8, in_=dmcol_in[:, :])
        v0c = cvec[:, 0:1]
        prc = cvec[:, 1:2]
        ompc = cvec[:, 2:3]
        # register const bias columns used by activation(bias=float)
        for ci, cval in enumerate((0.0, 1e-9, 1e-5)):
            cc = consts.tile([128, 1], f32, name=f"cc{ci}", tag=f"cc{ci}")
            nc.vector.memset(cc, cval)
            nc.const_aps.aps[(f32, cval)] = cc[:, :]

        # u broadcast row (every partition = full u vector)
        urow = big.tile([128, S], f32)
        nc.sync.dma_start(
            out=urow,
            in_=bass.AP(tensor=uvec_in[:].tensor, offset=uvec_in[:].offset,
                        ap=[[0, 128], [1, S]]),
        )

        # ---------------- DRAM scratch ----------------
        xb_d = dram.tile([S, D], bf16)          # normalized x, bf16
        snext_d = dram.tile([S], f32)
        sprev_d = dram.tile([S], f32)
        cum_d = dram.tile([S], f32)
        dsup_d = dram.tile([S + 1], f32)        # [0]=0, [1+i]=d_sup[i]
        colpack_d = dram.tile([HALF, 4], f32)   # per-row cols: cum,-cum,dSub,dSup

        # big SBUF residents
        xnt = xnt_pool.tile([128, 8, S], bf16)  # xnt[p,ft,i] = xn[i, ft*128+p]
        zball = zb_pool.tile([128, 8, S], bf16)  # zball[p,et,i] = zb[et*128+p, i]

        def emit_nb_tile(nbpool, t):
            r0 = t * 128
            w0 = 0 if t == 0 else r0 - 1
            nbt = nbpool.tile([128, S], f32, tag="nbt", name=f"nb{t}")
            nc.vector.tensor_scalar(
                nbt, urow, ucol8[:, t:t + 1], v0c, OP.mult, OP.add
            )
            if w0 > 0:
                nc.sync.dma_start(out=out_nb[r0:r0 + 128, 0:w0], in_=nbt[:, 0:w0])
            nc.sync.dma_start(
                out=out_nb[r0:r0 + 128, w0 + WB:S], in_=nbt[:, w0 + WB:S]
            )

        # ============ phase 1: LN + cast + transpose halves ============
        with nc.named_scope("p1_ln"):
            for g2 in range(8):
                xg = xgs[g2]
                xbg = xbpool.tile([128, 2, D], bf16, tag="xbg", name=f"xb{g2}")
                for j in range(2):
                    xt = xg[:, j, :]
                    # LN stats on the otherwise-idle ACT via accumulate
                    # outputs: sum(x) through Copy, sum(x^2) through Square.
                    dumpA = dpool.tile([128, D], bf16, tag="dumpA")
                    smc = stpool.tile([128, 1], f32, tag="smc")
                    nc.scalar.activation(dumpA, xt, AF.Copy, accum_out=smc)
                    dumpB = dpool.tile([128, D], bf16, tag="dumpB")
                    ssq = stpool.tile([128, 1], f32, tag="ssq")
                    nc.scalar.activation(dumpB, xt, AF.Square, accum_out=ssq)
                    mu = stpool.tile([128, 1], f32, tag="mu")
                    nc.vector.tensor_scalar(
                        mu, smc, 1.0 / D, None, OP.mult
                    )
                    var = stpool.tile([128, 1], f32, tag="var")
                    nc.vector.tensor_scalar(
                        var, ssq, 1.0 / D, None, OP.mult
                    )
                    mu2 = stpool.tile([128, 1], f32, tag="mu2")
                    nc.vector.tensor_tensor(mu2, mu, mu, OP.mult)
                    nc.vector.tensor_tensor(var, var, mu2, OP.subtract)
                    sq = stpool.tile([128, 1], f32, tag="sq")
                    nc.scalar.activation(sq, var, AF.Sqrt, bias=1e-5)
                    rstd = stpool.tile([128, 1], f32, tag="rstd")
                    nc.vector.reciprocal(rstd, sq)
                    nc.vector.tensor_scalar(
                        xbg[:, j, :], xt, mu, rstd,
                        OP.subtract, OP.mult,
                    )
                nc.sync.dma_start(
                    out=xb_d[g2 * 256:(g2 + 1) * 256, :].rearrange(
                        "(t p) e -> p t e", p=128
                    ),
                    in_=xbg,
                )
                # transpose each half as soon as its xb rows are in DRAM
                if g2 in (3, 7):
                    h = g2 // 4
                    for ft in range(8):
                        nc.sync.dma_start(
                            out=xnt[:, ft, h * 1024:(h + 1) * 1024],
                            in_=xb_d[h * 1024:(h + 1) * 1024,
                                     ft * 128:(ft + 1) * 128],
                            transpose=True,
                        )
            p1pools.close()

        # ============ phase 2b: z matmuls (fp8 DoubleRow, per half) ============
        # Band products are accumulated over e-blocks in SBUF (bf16 adds on
        # the otherwise-idle DVE) so only one small ones-matmul pass remains
        # after the z PSUM pool closes.
        ptsum1 = ptp.tile([128, S], bf16, tag="ptsum1")
        ptsum2 = ptp.tile([128, S], bf16, tag="ptsum2")
        with nc.named_scope("p2b_matmul"):
            nbi = 0
            for h in range(2):
                for et in range(8):
                    psz = pszp.tile([128, 1024], f32, tag="psz")
                    for ft in range(8):
                        lhs = at_sb[:, ft, et * 128:(et + 1) * 128]
                        for c in range(2):
                            off = h * 1024 + c * 512
                            nc.tensor.matmul(
                                psz[:, c * 512:(c + 1) * 512],
                                lhs,
                                xnt[:, ft, off:off + 512],
                                start=(ft == 0),
                                stop=(ft == 7),
                            )
                    nc.scalar.copy(
                        out=zball[:, et, h * 1024:(h + 1) * 1024], in_=psz
                    )
                    if h == 0:
                        # nb rank-1 tiles ride along on the idle DVE + DMA
                        if et % 2 == 1 and nbi < NT:
                            emit_nb_tile(nbpool, nbi)
                            nbi += 1
                    else:
                        # zb for this et is now complete -> band products
                        if et == 0:
                            nc.vector.tensor_tensor(
                                ptsum1[:, 0:S - 1], xnt[:, 0, 0:S - 1],
                                zball[:, 0, 1:S], OP.mult,
                            )
                            nc.vector.tensor_tensor(
                                ptsum2[:, 1:S], xnt[:, 0, 1:S],
                                zball[:, 0, 0:S - 1], OP.mult,
                            )
                        else:
                            pt1 = ptp.tile([128, S], bf16, tag="pt1")
                            nc.vector.tensor_tensor(
                                pt1[:, 0:S - 1], xnt[:, et, 0:S - 1],
                                zball[:, et, 1:S], OP.mult,
                            )
                            nc.vector.tensor_tensor(
                                ptsum1[:, 0:S - 1], ptsum1[:, 0:S - 1],
                                pt1[:, 0:S - 1], OP.add,
                            )
                            pt2 = ptp.tile([128, S], bf16, tag="pt2")
                            nc.vector.tensor_tensor(
                                pt2[:, 1:S], xnt[:, et, 1:S],
                                zball[:, et, 0:S - 1], OP.mult,
                            )
                            nc.vector.tensor_tensor(
                                ptsum2[:, 1:S], ptsum2[:, 1:S],
                                pt2[:, 1:S], OP.add,
                            )
                        if nbi < NT:
                            emit_nb_tile(nbpool, nbi)
                            nbi += 1
            for t in range(nbi, NT):
                emit_nb_tile(nbpool, t)
            p2bstack.close()

        # ============ phase 2c: partition reduce of the band products ============
        with nc.named_scope("p2c_reduce"), ExitStack() as p2cs:
            psnp = p2cs.enter_context(
                tc.tile_pool(name="psnp", bufs=1, space="PSUM")
            )
            rowp = p2cs.enter_context(tc.tile_pool(name="rowp", bufs=1))
            ps_n = psnp.tile([1, S], f32, tag="psn", name="ps_n")
            ps_p = psnp.tile([1, S], f32, tag="psp", name="ps_p")
            for c in range(4):
                nc.tensor.matmul(
                    ps_n[0:1, c * 512:(c + 1) * 512],
                    ones_b,
                    ptsum1[:, c * 512:(c + 1) * 512],
                    start=True, stop=True,
                )
            for c in range(4):
                nc.tensor.matmul(
                    ps_p[0:1, c * 512:(c + 1) * 512],
                    ones_b,
                    ptsum2[:, c * 512:(c + 1) * 512],
                    start=True, stop=True,
                )
            row_n = rowp.tile([1, S], f32, tag="rn")
            nc.scalar.mul(row_n, ps_n[0:1, :], 1.0 / 512.0)
            nc.sync.dma_start(out=snext_d[:], in_=row_n)
            row_p = rowp.tile([1, S], f32, tag="rp")
            nc.scalar.mul(row_p, ps_p[0:1, :], 1.0 / 512.0)
            nc.sync.dma_start(out=sprev_d[:], in_=row_p)

        # ============ phase 3: band math in [128,16] layout ============
        def v16(name):
            return vec.tile([128, 16], f32, tag="v16", name=name)

        def rd16(dtensor, off):  # dram vec [off:off+2048] -> [128,16] row-major
            return dtensor[off:off + S].rearrange("(p c) -> p c", c=16)

        with nc.named_scope("p3_band"):
            sn = v16("sn")
            nc.sync.dma_start(out=sn, in_=rd16(snext_d, 0))
            sp = v16("sp")
            nc.sync.dma_start(out=sp, in_=rd16(sprev_d, 0))
            hn_i = vec.tile([128, 16], i32, tag="v16i", name="hn_i")
            nc.sync.dma_start(out=hn_i, in_=rd16(hn_in[:], 0))
            hp_i = vec.tile([128, 16], i32, tag="v16i", name="hp_i")
            nc.sync.dma_start(out=hp_i, in_=rd16(hp_in[:], 0))
            cbi = vec.tile([128, 16], i32, tag="v16i", name="cbi")
            nc.sync.dma_start(out=cbi, in_=rd16(cbi_in[:], 0))
            cbS = v16("cbS")
            nc.sync.dma_start(out=cbS, in_=rd16(cbs_in[:], 0))
            wpv = v16("wpv")
            nc.sync.dma_start(out=wpv, in_=rd16(wpv_in[:], 0))
            wmv = v16("wmv")
            nc.sync.dma_start(out=wmv, in_=rd16(wmv_in[:], 0))
            neg9 = v16("neg9")
            nc.vector.memset(neg9, -1.0e9)

            # scores are tiny (|s| <~ 0.3) so exp never overflows; masked
            # entries are exp(-1e9)=0.  caseB rows would give 0/0 -> add the
            # cb flag to the denominator (their sm value is fixed by the
            # blend below anyway).
            sne = v16("sne")
            nc.vector.select(sne, hn_i, sn, neg9)
            spe = v16("spe")
            nc.vector.select(spe, hp_i, sp, neg9)
            en = v16("en")
            nc.scalar.activation(en, sne, AF.Exp)
            ep = v16("ep")
            nc.scalar.activation(ep, spe, AF.Exp)
            zs = v16("zs")
            nc.vector.tensor_tensor(zs, en, ep, OP.add)
            rz = v16("rz")
            nc.vector.reciprocal(rz, zs)
            # caseB rows give 0*inf=NaN here; the select replaces them
            # with the uniform 1/S value.
            nn = v16("nn")
            nc.vector.tensor_tensor(nn, en, rz, OP.mult)
            nc.vector.select(nn, cbi, cbS, nn)
            npv = v16("npv")
            nc.vector.tensor_tensor(npv, ep, rz, OP.mult)
            nc.vector.select(npv, cbi, cbS, npv)
            # np shifted by +1 (value at i+1)
            npsh = v16("npsh")
            nc.vector.memset(npsh, 0.0)
            nc.vector.tensor_copy(out=npsh[:, 0:15], in_=npv[:, 1:16])
            nc.sync.dma_start(out=npsh[0:127, 15:16], in_=npv[1:128, 0:1])
            msup = v16("msup")
            nc.vector.tensor_tensor(msup, nn, npsh, OP.mult)
            # d_sup = prior + (1-prior)*sqrt(msup+1e-9)
            dsup = v16("dsup")
            nc.scalar.activation(dsup, msup, AF.Sqrt, bias=1e-9)
            nc.vector.tensor_scalar(dsup, dsup, ompc, prc, OP.mult, OP.add)
            # ell, prefix sums
            ell = v16("ell")
            nc.scalar.activation(ell, dsup, AF.Ln, bias=1e-9)
            zv16 = v16("zv16")
            nc.vector.memset(zv16, 0.0)
            incl = v16("incl")
            nc.vector.tensor_tensor_scan(incl, ell, zv16, 0.0, OP.add, OP.add)
            excl = v16("excl")
            nc.vector.tensor_tensor(excl, incl, ell, OP.subtract)
            with ExitStack() as p3s:
                ps3p = p3s.enter_context(
                    tc.tile_pool(name="ps3p", bufs=1, space="PSUM")
                )
                ps3 = ps3p.tile([128, 512], f32)
                nc.tensor.matmul(
                    ps3[:, 0:1], lt128, incl[:, 15:16], start=True, stop=True
                )
                cp_col = vec.tile([128, 1], f32, tag="cpc", name="cp_col")
                nc.vector.tensor_copy(out=cp_col, in_=ps3[:, 0:1])
            cum = v16("cum")
            nc.vector.tensor_scalar(cum, excl, cp_col, None, OP.add)
            ncum = v16("ncum")
            nc.vector.tensor_scalar(ncum, cum, -1.0, None, OP.mult)

            def wr16(dtensor, off, src):
                nc.sync.dma_start(
                    out=dtensor[off:off + S].rearrange("(p c) -> p c", c=16),
                    in_=src,
                )

            wr16(cum_d, 0, cum)
            wr16(dsup_d, 1, dsup)
            z1 = vec.tile([1, 1], f32, tag="z1", name="z1")
            nc.vector.memset(z1, 0.0)
            nc.sync.dma_start(out=dsup_d[0:1], in_=z1)
            dsupsh = v16("dsupsh")   # dsup[i-1]
            nc.sync.dma_start(out=dsupsh, in_=rd16(dsup_d, 0))
            dsub = v16("dsub")       # delta for subdiagonal of nb
            nc.vector.tensor_tensor(dsub, dsupsh, wmv, OP.subtract)
            dsp = v16("dsp")         # delta for superdiagonal of nb
            nc.vector.tensor_tensor(dsp, dsup, wpv, OP.subtract)

            # pack per-row columns -> DRAM -> [128, 8, 4] col-major reload
            pack = vec.tile([128, 16, 4], f32, tag="pack", name="pack")
            nc.vector.tensor_copy(out=pack[:, :, 0], in_=cum)
            nc.vector.tensor_copy(out=pack[:, :, 1], in_=ncum)
            nc.vector.tensor_copy(out=pack[:, :, 2], in_=dsub)
            nc.vector.tensor_copy(out=pack[:, :, 3], in_=dsp)
            nc.sync.dma_start(
                out=colpack_d[0:HALF, :].rearrange("(p c) q -> p c q", p=64),
                in_=pack[0:64, :, :],
            )
            cp_all = big.tile([128, 8, 4], f32)
            nc.sync.dma_start(
                out=cp_all,
                in_=colpack_d[0:HALF, :].rearrange("(t p) q -> p t q", p=128),
            )
            cumrow = big.tile([128, S], f32)
            nc.sync.dma_start(
                out=cumrow,
                in_=bass.AP(tensor=cum_d[:].tensor, offset=cum_d[:].offset,
                            ap=[[0, 128], [1, S]]),
            )

        # ============ phase 4: g tiles + nb band windows ============
        with nc.named_scope("p4_out"):
            with ExitStack() as p4:
                gp = p4.enter_context(tc.tile_pool(name="gp", bufs=3))
                winp = p4.enter_context(tc.tile_pool(name="winp", bufs=3))
                bwp = p4.enter_context(tc.tile_pool(name="bwp", bufs=3))

                for t in range(NT):
                    r0 = t * 128
                    w0 = 0 if t == 0 else r0 - 1
                    cum_c = cp_all[:, t, 0:1]
                    ncum_c = cp_all[:, t, 1:2]
                    dsub_c = cp_all[:, t, 2:3]
                    dsp_c = cp_all[:, t, 3:4]

                    # g tile: exp(cum_j - cum_i) off-diag (cum non-increasing
                    # -> both triangles are exp(-|delta|)), diag dmain via
                    # eye*(dmain-1) add; +1e-9 via max floor (exp underflow
                    # lands exactly on 1e-9).  Ship each region as it's done.
                    g = gp.tile([128, S], f32, tag="g", name=f"g{t}")
                    if t > 0:
                        nc.scalar.activation(
                            g[:, 0:r0], cumrow[:, 0:r0], AF.Exp,
                            bias=cum_c, scale=-1.0,
                        )
                        nc.vector.tensor_scalar(
                            g[:, 0:r0], g[:, 0:r0], 1e-9, None, OP.max
                        )
                        nc.sync.dma_start(
                            out=out_g[r0:r0 + 128, 0:r0], in_=g[:, 0:r0]
                        )
                    nc.scalar.activation(
                        g[:, r0 + 128:S], cumrow[:, r0 + 128:S], AF.Exp,
                        bias=ncum_c, scale=1.0,
                    )
                    nc.vector.tensor_scalar(
                        g[:, r0 + 128:S], g[:, r0 + 128:S], 1e-9, None, OP.max
                    )
                    nc.sync.dma_start(
                        out=out_g[r0:r0 + 128, r0 + 128:S],
                        in_=g[:, r0 + 128:S],
                    )
                    # exp(-|w|) = min(exp(w), exp(-w)); inf from overflow is
                    # harmless under min.
                    w = winp.tile([128, 128], f32, tag="w", name=f"w{t}")
                    nc.vector.tensor_scalar(
                        w, cumrow[:, r0:r0 + 128], cum_c, None, OP.subtract
                    )
                    e1 = winp.tile([128, 128], f32, tag="e1", name=f"e1_{t}")
                    nc.scalar.activation(e1, w, AF.Exp)
                    e2 = winp.tile([128, 128], f32, tag="e2", name=f"e2_{t}")
                    nc.scalar.activation(e2, w, AF.Exp, scale=-1.0)
                    nc.vector.tensor_tensor(
                        g[:, r0:r0 + 128], e1, e2, OP.min
                    )
                    eyed = winp.tile([128, 128], f32, tag="eyed", name=f"ey{t}")
                    nc.vector.tensor_scalar(
                        eyed, eye_sb, dmcol8[:, t:t + 1], None, OP.mult
                    )
                    nc.vector.tensor_tensor(
                        g[:, r0:r0 + 128], g[:, r0:r0 + 128], eyed, OP.add
                    )
                    nc.vector.tensor_scalar(
                        g[:, r0:r0 + 128], g[:, r0:r0 + 128], 1e-9, None,
                        OP.max,
                    )
                    nc.sync.dma_start(
                        out=out_g[r0:r0 + 128, r0:r0 + 128],
                        in_=g[:, r0:r0 + 128],
                    )

                    # nb band window [r0, w0:w0+WB]
                    bw = bwp.tile([128, WB], f32, tag="bw", name=f"bw{t}")
                    nc.vector.tensor_scalar(
                        bw, urow[:, w0:w0 + WB], ucol8[:, t:t + 1], v0c,
                        OP.mult, OP.add,
                    )
                    v = 0 if t == 0 else 1
                    tsub = bwp.tile([128, WB], f32, tag="tsub", name=f"ts{t}")
                    nc.vector.tensor_scalar(
                        tsub, bm_sb[:, 2 * v + 0, :], dsub_c, None, OP.mult
                    )
                    nc.vector.tensor_tensor(bw, bw, tsub, OP.add)
                    tsup = bwp.tile([128, WB], f32, tag="tsup", name=f"tp{t}")
                    nc.vector.tensor_scalar(
                        tsup, bm_sb[:, 2 * v + 1, :], dsp_c, None, OP.mult
                    )
                    nc.vector.tensor_tensor(bw, bw, tsup, OP.add)
                    nc.sync.dma_start(
                        out=out_nb[r0:r0 + 128, w0:w0 + WB], in_=bw
                    )

    nc.compile()
    return nc


def _consts():
    k = np.arange(128)
    lt = (k[:, None] < k[None, :]).astype(np.float32)       # lt[k,p]=k<p
    eye = (k[None, :] == k[:, None]).astype(np.float32)
    # band masks [variant, 128, WB]: variant 0 -> t=0 (w0=0), 1 -> t>0 (w0=r0-1)
    w = np.arange(WB)
    bm = np.zeros((4, 128, WB), np.float32)
    bm[0][(w[None, :] == k[:, None] - 1)] = 1.0   # sub,  t=0 (absent for p=0)
    bm[1][(w[None, :] == k[:, None] + 1)] = 1.0   # sup,  t=0
    bm[2][(w[None, :] == k[:, None])] = 1.0       # sub,  t>0
    bm[3][(w[None, :] == k[:, None] + 2)] = 1.0   # sup,  t>0
    import ml_dtypes
    ones = np.ones((128, 1), dtype=ml_dtypes.bfloat16)
    return lt, eye, bm, ones


def kernel(context, eos_mask, prior, wq, bq, wk, bk, gamma, beta):
    import ml_dtypes
    from concourse.bass_utils import run_bass_kernel_spmd

    if "nc" not in _cache:
        _cache["nc"] = _build()
    nc = _cache["nc"]

    context = np.asarray(context, np.float32)
    eos_mask = np.asarray(eos_mask, np.int32)
    prior = float(np.asarray(prior, np.float32).reshape(-1)[0])
    wq = np.asarray(wq, np.float32)
    wk = np.asarray(wk, np.float32)
    lt, eye, bm, ones = _consts()

    # at[p, ft, e] = (wk^T wq)[ft*128+p, e]
    M = (wk.T @ wq).astype(np.float32)
    at = np.ascontiguousarray(
        M.reshape(8, 128, D).transpose(1, 0, 2)
    ).astype(ml_dtypes.bfloat16)

    p32 = np.float32(prior)
    omp = np.float32(1.0) - p32
    v0 = np.float32(p32 + omp * np.float32(C_SQ9))
    vbb = np.float32(p32 + omp * np.float32(C_SBB))
    dv = np.float32(vbb - v0)
    cvec = np.zeros((128, 4), np.float32)
    cvec[:, 0] = v0
    cvec[:, 1] = p32
    cvec[:, 2] = omp

    in_maps = []
    for c in range(8):
        b, h = c // 2, c % 2
        x = context[b] if h == 0 else context[b][::-1]
        eo = eos_mask[b] if h == 0 else eos_mask[b][::-1]
        hn = np.zeros(S, np.int32)
        hn[:S - 1] = eo[1:]
        hp = np.zeros(S, np.int32)
        hp[1:] = eo[:S - 1]
        cb = ((hn == 0) & (hp == 0)).astype(np.float32)
        cbi = ((hn == 0) & (hp == 0)).astype(np.int32)
        cbs = (cb * np.float32(1.0 / S)).astype(np.float32)
        uscl = (dv * cb).astype(np.float32)
        un = np.zeros(S, np.float32)
        un[:S - 1] = cb[1:]
        up = np.zeros(S, np.float32)
        up[1:] = cb[:S - 1]
        wpv = (v0 + uscl * un).astype(np.float32)
        wmv = (v0 + uscl * up).astype(np.float32)
        dmain = (v0 + dv * cb).astype(np.float32)
        ucol8 = np.ascontiguousarray(uscl[:HALF].reshape(8, 128).T)
        # dmcol carries (dmain - 1): g window diag = exp(0) + (dmain-1)
        dmcol8 = np.ascontiguousarray((dmain[:HALF] - 1.0).reshape(8, 128).T
                                      .astype(np.float32))
        in_maps.append({
            "x": np.ascontiguousarray(x).astype(ml_dtypes.bfloat16),
            "at": at,
            "hn": hn, "hp": hp,
            "uvec": cb, "cbi": cbi, "cbs": cbs,
            "wpv": wpv, "wmv": wmv,
            "ucol": ucol8, "dmcol": dmcol8,
            "cvec": cvec,
            "lt128": lt, "eye128": eye,
            "bmasks": bm, "onesb": ones,
        })

    bkr = run_bass_kernel_spmd(nc, in_maps, core_ids=list(range(8)))
    _cache["last_bkr"] = bkr

    g_out = np.empty((B, S, S), np.float32)
    nb_out = np.empty((B, S, S), np.float32)
    for c in range(8):
        b, h = c // 2, c % 2
        rg = bkr.results[c]["out_g"]
        rn = bkr.results[c]["out_nb"]
        if h == 0:
            g_out[b, :HALF] = rg
            nb_out[b, :HALF] = rn
        else:
            g_out[b, HALF:] = rg[::-1, ::-1]
            nb_out[b, HALF:] = rn[::-1, ::-1]
    return g_out, nb_out
